# revision 9
# baseline (speedup 1.0000x reference)
"""Policy-masked sparse attention on 8 trn2 NeuronCores.

Strategy (data-parallel over B: one batch element per core):
  Reference softmax-with-policy (eps=1e-6) reduces, for this regime, to:
    - dropped queries (policy=0): out row = v_row @ Wproj + b  (x @ W2)
    - kept queries: out = (E @ V) / (E @ 1) over kept keys, E = exp(S)
  Host compacts kept/dropped tokens, pads kept to NK (mult of 256) and
  queries to NKM (mult of 32), pre-scales + fp8-casts weights.

  Device (per core):
    - QKV + V and the two projections run as fp8(e4m3) DoubleRow matmuls
      (contraction 256 per pass) — ~1.7x PE throughput vs fp16.
    - S^T = K^T.T @ Q^T per head (fp16, contraction 64), exp on ScalarE
      with scale=1/8192 folded in and a per-partition bias that kills
      padded keys (exp -> 0), output directly as fp8 into the DoubleRow
      slot layout.
    - T^T accumulation uses an augmented V: per head 64 v-columns + 64
      constant 0.5-columns, so PSUM rows 64:128 hold the softmax
      denominator replicated 64x. reciprocal_approx_fast on [64, NKM]
      then one tensor_tensor multiply writes the normalized attention
      output as fp8 (x16 scale) in proj DoubleRow layout. No gpsimd
      broadcast, no ScalarE copies.
    - proj is emitted output-transposed (out^T[c, t]) so the bias is a
      per-partition scalar folded into the DVE epilogue; host transposes.
  Input DMAs are merged into 9 large descriptors, K-weights + x first.
"""

import math
import numpy as np
import ml_dtypes

import concourse.bass as bass
import concourse.bacc as bacc
import concourse.mybir as mybir
from concourse import tile
from concourse.bass_utils import run_bass_kernel_spmd

C = 768
H = 12
HD = 64
CB = C // 128           # feature blocks of 128
CBP = CB // 2           # feature pair-blocks of 256 (DoubleRow)
F16 = mybir.dt.float16
F32 = mybir.dt.float32
F8 = mybir.dt.float8e4
DR = mybir.MatmulPerfMode.DoubleRow
NPF8 = ml_dtypes.float8_e4m3

# fp8 range scales (see docstring)
AQ = 32.0               # Wq (with 1/sqrt(hd) folded) scale
AK = 32.0               # Wk scale
AV = 8.0                # Wv scale
AP_ = 32.0              # Wproj scale
ONES = 0.5              # denominator ones-column value
S_SCALE = 1.0 / (AQ * AK * 8.0)   # S_psum -> true S (1/sqrt(64) fold in AQ)
O_SCALE = AV / ONES               # OAT = O_SCALE * O_true
PROJ_SCALE = 1.0 / (O_SCALE * AP_)
KILL = -30000.0

_cache = {}


def _groups(n, limit=512):
    out = []
    off = 0
    while off < n:
        g = min(limit, n - off)
        out.append((off, g))
        off += g
    return out


def _build(NK, ND, NKM):
    """Build + bacc-compile the 8-core SPMD program for padded sizes."""
    KB = NK // 128
    KBP = KB // 2
    nc = bacc.Bacc("TRN2", target_bir_lowering=False, debug=False,
                   num_devices=8)

    wk8 = nc.dram_tensor("wk8", [CBP * 128, 2 * C], F8, kind="ExternalInput").ap()
    wq8 = nc.dram_tensor("wq8", [CBP * 128, 2 * C], F8, kind="ExternalInput").ap()
    wv8 = nc.dram_tensor("wv8", [CBP * 128, 2 * C], F8, kind="ExternalInput").ap()
    xc8 = nc.dram_tensor("xc8", [CBP * 128, 2 * NK], F8, kind="ExternalInput").ap()
    wp8 = nc.dram_tensor("wp8", [CBP * 128, 2 * C], F8, kind="ExternalInput").ap()
    xdT = nc.dram_tensor("xdT", [C, ND], F16, kind="ExternalInput").ap()
    w2T = nc.dram_tensor("w2T", [C, C], F16, kind="ExternalInput").ap()
    biasT = nc.dram_tensor("biasT", [128, CB], F32, kind="ExternalInput").ap()
    killT = nc.dram_tensor("killT", [128, KB], F32, kind="ExternalInput").ap()
    outkT = nc.dram_tensor("outkT", [C, NKM], F16, kind="ExternalOutput").ap()
    outdT = nc.dram_tensor("outdT", [C, ND], F16, kind="ExternalOutput").ap()
    import os
    DBG = bool(os.environ.get("KDBG"))
    if DBG:
        dq0 = nc.dram_tensor("dq0", [128, NKM], F16, kind="ExternalOutput").ap()
        dk0 = nc.dram_tensor("dk0", [128, NK], F16, kind="ExternalOutput").ap()
        de0 = nc.dram_tensor("de0", [128, 2 * NKM], F8, kind="ExternalOutput").ap()
        dv0 = nc.dram_tensor("dv0", [128, H * 256], F8, kind="ExternalOutput").ap()
        do0 = nc.dram_tensor("do0", [128, 2 * NKM], F8, kind="ExternalOutput").ap()

    GK = _groups(NK)
    GKM = _groups(NKM)
    GC = _groups(C)

    with tile.TileContext(nc) as tc:
        with (
            tc.tile_pool(name="const", bufs=1) as cpool,
            tc.tile_pool(name="ins", bufs=1) as ipool,
            tc.tile_pool(name="acts", bufs=1) as apool,
            tc.tile_pool(name="work", bufs=4) as wpool,
            tc.tile_pool(name="outs", bufs=3) as opool,
            tc.tile_pool(name="ps", bufs=2, space="PSUM") as pspool,
            tc.tile_pool(name="pt", bufs=2, space="PSUM") as ptpool,
        ):
            # ---- merged inputs (DMA emission order = priority) ----
            def dma_blocked(dst, src, nb):
                nc.sync.dma_start(
                    dst[:].rearrange("p (b c) -> p b c", b=nb),
                    src.rearrange("(b p) c -> p b c", p=128))

            wk_t = ipool.tile([128, CBP * 2 * C], F8, name="wk", tag="wk")
            xc_t = ipool.tile([128, CBP * 2 * NK], F8, name="xc", tag="xc")
            for bp in range(CBP):
                nc.sync.dma_start(wk_t[:, bp * 2 * C:(bp + 1) * 2 * C],
                                  wk8[bp * 128:(bp + 1) * 128, :])
                nc.sync.dma_start(xc_t[:, bp * 2 * NK:(bp + 1) * 2 * NK],
                                  xc8[bp * 128:(bp + 1) * 128, :])
            wqt_t = ipool.tile([128, CBP * 2 * C], F8, name="wq", tag="wq")
            dma_blocked(wqt_t, wq8, CBP)
            wv_t = ipool.tile([128, CBP * 2 * C], F8, name="wv", tag="wv")
            dma_blocked(wv_t, wv8, CBP)
            kill_t = cpool.tile([128, KB], F32, name="kill", tag="kill")
            nc.sync.dma_start(kill_t[:], killT[:])
            xd_t = ipool.tile([128, CB * ND], F16, name="xd", tag="xd")
            dma_blocked(xd_t, xdT, CB)
            w2_t = ipool.tile([128, CB * C], F16, name="w2", tag="w2")
            dma_blocked(w2_t, w2T, CB)
            wp_t = ipool.tile([128, CBP * 2 * C], F8, name="wp", tag="wp")
            dma_blocked(wp_t, wp8, CBP)
            bias_t = cpool.tile([128, CB], F32, name="bias", tag="bias")
            nc.sync.dma_start(bias_t[:], biasT[:])

            def w_slice(t, bp, j):
                """[128, 2, 128] DoubleRow lhsT slice: pair-block bp, col
                chunk j of a [128, CBP*2*C] fp8 weight tile."""
                return t[:, bp * 2 * C:(bp + 1) * 2 * C].rearrange(
                    "p (s c) -> p s c", s=2)[:, :, j * 128:(j + 1) * 128]

            def x_slice(bp, o, n):
                return xc_t[:, bp * 2 * NK:(bp + 1) * 2 * NK].rearrange(
                    "p (s c) -> p s c", s=2)[:, :, o:o + n]

            def xw_slice(bp, tb):
                """x as DoubleRow lhsT for the V (token-major) matmul."""
                return xc_t[:, bp * 2 * NK:(bp + 1) * 2 * NK].rearrange(
                    "p (s c) -> p s c", s=2)[:, :, tb * 128:(tb + 1) * 128]

            def wv_rhs(bp, o, n):
                return wv_t[:, bp * 2 * C:(bp + 1) * 2 * C].rearrange(
                    "p (s c) -> p s c", s=2)[:, :, o:o + n]

            # ---- persistent intermediates ----
            QcT = [apool.tile([128, NKM], F16, name=f"q{j}", tag=f"q{j}")
                   for j in range(CB)]
            KcT = [apool.tile([128, NK], F16, name=f"k{j}", tag=f"k{j}")
                   for j in range(CB)]
            Vag = [apool.tile([128, H * 256], F8, name=f"va{kp}",
                              tag=f"va{kp}") for kp in range(KBP)]
            for kp in range(KBP):
                va4 = Vag[kp][:].rearrange("p (h s m) -> p h s m", s=2, m=128)
                nc.gpsimd.memset(va4[:, :, :, 0:HD], ONES)
            OAT = [apool.tile([128, 2 * NKM], F8, name=f"oat{fp}",
                              tag=f"oat{fp}") for fp in range(CBP)]
            ET = {}
            for hm in range(6):
                for kp in range(KBP):
                    ET[(hm, kp)] = apool.tile(
                        [128, 2 * NKM], F8, name=f"et{hm}_{kp}",
                        tag=f"et{hm}_{kp}")

            def qkv_units(j):
                """f-major chunk j of Wq/Wk as 4 emission units."""
                grps = GKM if j < CB else GK
                w = NKM if j < CB else NK
                wt = wqt_t if j < CB else wk_t
                state = {}

                def mm(bp):
                    if bp == 0:
                        state["ps"] = pspool.tile([128, w], F32, name="qps",
                                                  tag="s")
                    ps = state["ps"]
                    for (o, n) in grps:
                        nc.tensor.matmul(
                            ps[:, o:o + n],
                            lhsT=w_slice(wt, bp, j % CB),
                            rhs=x_slice(bp, o, n),
                            start=(bp == 0), stop=(bp == CBP - 1),
                            perf_mode=DR)

                def cast():
                    dest = QcT[j] if j < CB else KcT[j - CB]
                    nc.vector.tensor_copy(dest[:, 0:w], state["ps"][:, 0:w])

                return [lambda bp=bp: mm(bp) for bp in range(CBP)] + [cast]

            def v_units(tb):
                """token-major V chunk for kept block tb as 4 units."""
                state = {}

                def mm(bp):
                    if bp == 0:
                        state["ps"] = pspool.tile([128, C], F32, name="vps",
                                                  tag="s")
                    ps = state["ps"]
                    for (o, n) in GC:
                        nc.tensor.matmul(
                            ps[:, o:o + n],
                            lhsT=xw_slice(bp, tb),
                            rhs=wv_rhs(bp, o, n),
                            start=(bp == 0), stop=(bp == CBP - 1),
                            perf_mode=DR)

                def cast():
                    va4 = Vag[tb // 2][:].rearrange(
                        "p (h s m) -> p h s m", s=2, m=128)
                    ps3 = state["ps"][:].rearrange("p (h d) -> p h d", d=HD)
                    nc.vector.tensor_copy(va4[:, :, tb % 2, HD:128], ps3)

                return [lambda bp=bp: mm(bp) for bp in range(CBP)] + [cast]

            def s_exp_kb_h(p, kb, hh):
                """S^T then exp->fp8 for head 2p+hh at key block kb."""
                fc = p
                h = 2 * p + hh
                rows = slice(hh * 64, hh * 64 + 64)
                et = ET[(h % 6, kb // 2)]
                ps = pspool.tile([128, NKM], F32, name="sps", tag="s")
                for (o, n) in GKM:
                    nc.tensor.matmul(
                        ps[:, o:o + n],
                        lhsT=KcT[fc][rows, kb * 128:(kb + 1) * 128],
                        rhs=QcT[fc][rows, o:o + n],
                        start=True, stop=True)
                nc.scalar.activation(
                    et[:, (kb % 2) * NKM:(kb % 2) * NKM + NKM], ps[:],
                    mybir.ActivationFunctionType.Exp,
                    bias=kill_t[:, kb:kb + 1], scale=S_SCALE)

            def proj_kept(cb):
                """out^T[c-block cb, :] = (OAT @ wp) * PROJ_SCALE + bias."""
                ps = pspool.tile([128, NKM], F32, name="pps", tag="s")
                for fp in range(CBP):
                    oat3 = OAT[fp][:].rearrange("p (s t) -> p s t", s=2)
                    for (o, n) in GKM:
                        nc.tensor.matmul(
                            ps[:, o:o + n],
                            lhsT=w_slice(wp_t, fp, cb),
                            rhs=oat3[:, :, o:o + n],
                            start=(fp == 0), stop=(fp == CBP - 1),
                            perf_mode=DR)
                ok = opool.tile([128, NKM], F16, name="ok", tag="ok")
                nc.vector.tensor_scalar(
                    ok[:], ps[:], PROJ_SCALE, bias_t[:, cb:cb + 1],
                    op0=mybir.AluOpType.mult, op1=mybir.AluOpType.add)
                nc.sync.dma_start(outkT[cb * 128:(cb + 1) * 128, :], ok[:])

            def proj_drop(cb):
                """out^T[c-block cb, :] = x_d @ W2^T + bias (fp16)."""
                ps = pspool.tile([128, ND], F32, name="dps", tag="s")
                for fb in range(CB):
                    nc.tensor.matmul(
                        ps[:],
                        lhsT=w2_t[:, fb * C + cb * 128:fb * C + (cb + 1) * 128],
                        rhs=xd_t[:, fb * ND:(fb + 1) * ND],
                        start=(fb == 0), stop=(fb == CB - 1))
                ok = opool.tile([128, ND], F16, name="od", tag="od")
                nc.vector.tensor_scalar(
                    ok[:], ps[:], bias_t[:, cb:cb + 1], None,
                    op0=mybir.AluOpType.add)
                nc.sync.dma_start(outdT[cb * 128:(cb + 1) * 128, :], ok[:])

            # ---- T^T accumulate + normalize, split into emission units ----
            # PSUM rows 0:64 hold the replicated denominator (ones block
            # first in Vag), rows 64:128 hold AV*T: reciprocal_approx_fast
            # reads PSUM at base partition 0 (base-64 custom reads are
            # broken), the multiply's in0 may be PSUM at base 64.
            def tpair_units(p):
                state = {}

                def accum(hh, kp):
                    h = 2 * p + hh
                    if kp == 0:
                        state[hh] = ptpool.tile([128, NKM], F32, name="ptT",
                                                tag="t2")
                    ptT = state[hh]
                    va = Vag[kp][:, h * 256:(h + 1) * 256].rearrange(
                        "p (s m) -> p s m", s=2)
                    et3 = ET[(h % 6, kp)][:].rearrange(
                        "p (s t) -> p s t", s=2)
                    for (o, n) in GKM:
                        nc.tensor.matmul(
                            ptT[:, o:o + n],
                            lhsT=va,
                            rhs=et3[:, :, o:o + n],
                            start=(kp == 0), stop=(kp == KBP - 1),
                            perf_mode=DR)

                def chain(hh):
                    h = 2 * p + hh
                    fp = h // 4
                    sl = (h // 2) % 2
                    orow = (h % 2) * 64
                    ptT = state[hh]
                    rb = wpool.tile([64, NKM], F32, name="rb", tag="rb")
                    nc.vector.reciprocal_approx_fast(rb[:], ptT[0:64, :])
                    nc.vector.tensor_tensor(
                        OAT[fp][orow:orow + 64, sl * NKM:sl * NKM + NKM],
                        ptT[64:128, :], rb[:], op=mybir.AluOpType.mult)

                units = []
                for hh in range(2):
                    units += [lambda hh=hh, kp=kp: accum(hh, kp)
                              for kp in range(KBP)]
                    units.append(lambda hh=hh: chain(hh))
                return units

            # ---- schedule ----
            # S/exp is the rate-limiting stream (ScalarE); other PE work is
            # woven between S slots in fine units so the PE never stalls
            # long (HAM stays at full clock). proj_drop runs in the tail so
            # its w2-DMA wait never blocks the PE FIFO behind early S work.
            NP = H // 2
            for f in qkv_units(CB + 0) + qkv_units(0):
                f()
            for p in range(NP):
                fillers = []
                if p + 1 < NP:
                    fillers += qkv_units(CB + p + 1) + qkv_units(p + 1)
                if p == 0:
                    for tb in range(KB):
                        fillers += v_units(tb)
                if p >= 1:
                    fillers += tpair_units(p - 1)
                nslots = 2 * KB
                done = 0
                slot = 0
                for kb in range(KB):
                    for hh in range(2):
                        s_exp_kb_h(p, kb, hh)
                        slot += 1
                        want = (len(fillers) * slot + nslots - 1) // nslots
                        while done < want:
                            fillers[done]()
                            done += 1
            tail = tpair_units(NP - 1)
            for cb in range(CB):
                tail.append(lambda cb=cb: proj_drop(cb))
            for f in tail:
                f()
            for cb in range(CB):
                proj_kept(cb)
            if DBG:
                nc.sync.dma_start(dq0[:], QcT[0][:])
                nc.sync.dma_start(dk0[:], KcT[0][:])
                nc.sync.dma_start(de0[:], ET[(0, 0)][:])
                nc.sync.dma_start(dv0[:], Vag[0][:])
                nc.sync.dma_start(do0[:], OAT[0][:])

    nc.compile()
    return nc


def kernel(x, policy, Wqkv, Wproj, bproj, _trace=False, _tmpdir=None):
    x = np.asarray(x)
    policy = np.asarray(policy)
    Wqkv = np.asarray(Wqkv, dtype=np.float32)
    Wproj = np.asarray(Wproj, dtype=np.float32)
    bproj = np.asarray(bproj, dtype=np.float32)
    B, N, _ = x.shape
    assert B == 8 and x.shape[2] == C

    pol = policy[:, :, 0] > 0.5
    kept = [np.nonzero(pol[b])[0] for b in range(B)]
    drop = [np.nonzero(~pol[b])[0] for b in range(B)]
    nk = [len(i) for i in kept]
    nd = [len(i) for i in drop]
    NK = max(256, int(math.ceil(max(nk) / 256.0)) * 256)
    ND = max(128, int(math.ceil(max(nd) / 128.0)) * 128)
    NKM = min(NK, max(128, int(math.ceil(max(nk) / 32.0)) * 32))
    KB = NK // 128
    assert NK - min(nk) <= 0x7FFF

    key = (NK, ND, NKM)
    if key not in _cache:
        _cache[key] = _build(NK, ND, NKM)
    nc = _cache[key]

    # ---- shared weight prep ----
    def dr_pack(wT, scale):
        # [C, cols] f-major -> DoubleRow pair layout [CBP*128, 2*cols]
        a = (wT * scale).astype(np.float32)
        cols = a.shape[1]
        a = a.reshape(CBP, 2, 128, cols).transpose(0, 2, 1, 3)
        return np.ascontiguousarray(a.reshape(CBP * 128, 2 * cols)).astype(NPF8)

    wqkvT = np.ascontiguousarray(Wqkv.T)           # [C, 3C]
    wq8a = dr_pack(wqkvT[:, 0:C], AQ)              # 1/sqrt(hd) lives in S_SCALE
    wk8a = dr_pack(wqkvT[:, C:2 * C], AK)
    wv8a = dr_pack(wqkvT[:, 2 * C:3 * C], AV)
    wp8a = dr_pack(np.ascontiguousarray(Wproj.T), AP_)
    W2 = Wproj @ Wqkv[2 * C:3 * C]
    w2Ta = np.ascontiguousarray(W2.T).astype(np.float16)
    biasa = np.ascontiguousarray(
        bproj.reshape(CB, 128).T).astype(np.float32)   # [128, CB]

    in_maps = []
    for b in range(B):
        xcT = np.zeros((C, NK), np.float32)
        xcT[:, :nk[b]] = x[b][kept[b]].T
        xc8a = np.ascontiguousarray(
            xcT.reshape(CBP, 2, 128, NK).transpose(0, 2, 1, 3)
            .reshape(CBP * 128, 2 * NK)).astype(NPF8)
        xdTa = np.zeros((C, ND), np.float16)
        xdTa[:, :nd[b]] = x[b][drop[b]].T
        killa = np.zeros((128, KB), np.float32)
        for kb in range(KB):
            lo = kb * 128
            for p_ in range(128):
                if lo + p_ >= nk[b]:
                    killa[p_, kb] = KILL
        in_maps.append({
            "wk8": wk8a, "wq8": wq8a, "wv8": wv8a, "xc8": xc8a,
            "wp8": wp8a, "xdT": xdTa, "w2T": w2Ta, "biasT": biasa,
            "killT": killa,
        })

    res = run_bass_kernel_spmd(nc, in_maps, core_ids=list(range(B)),
                               trace=_trace, tmpdir=_tmpdir)

    out = np.empty((B, N, C), np.float32)
    for b in range(B):
        out[b, kept[b]] = res.results[b]["outkT"][:, :nk[b]].T.astype(np.float32)
        out[b, drop[b]] = res.results[b]["outdT"][:, :nd[b]].T.astype(np.float32)
    kernel._last = res
    return out


# revision 11
# speedup vs baseline: 1.0138x; 1.0138x over previous
"""Policy-masked sparse attention on 8 trn2 NeuronCores.

Strategy (data-parallel over B: one batch element per core):
  Reference softmax-with-policy (eps=1e-6) reduces, for this regime, to:
    - dropped queries (policy=0): out row = v_row @ Wproj + b  (x @ W2)
    - kept queries: out = (E @ V) / (E @ 1) over kept keys, E = exp(S)
  Host compacts kept/dropped tokens, pads kept to NK (mult of 256) and
  queries to NKM (mult of 32), pre-scales + fp8-casts weights.

  Device (per core):
    - QKV + V and the two projections run as fp8(e4m3) DoubleRow matmuls
      (contraction 256 per pass) — ~1.7x PE throughput vs fp16.
    - S^T = K^T.T @ Q^T per head (fp16, contraction 64), exp on ScalarE
      with scale=1/8192 folded in and a per-partition bias that kills
      padded keys (exp -> 0), output directly as fp8 into the DoubleRow
      slot layout.
    - T^T accumulation uses an augmented V: per head 64 v-columns + 64
      constant 0.5-columns, so PSUM rows 64:128 hold the softmax
      denominator replicated 64x. reciprocal_approx_fast on [64, NKM]
      then one tensor_tensor multiply writes the normalized attention
      output as fp8 (x16 scale) in proj DoubleRow layout. No gpsimd
      broadcast, no ScalarE copies.
    - proj is emitted output-transposed (out^T[c, t]) so the bias is a
      per-partition scalar folded into the DVE epilogue; host transposes.
  Input DMAs are merged into 9 large descriptors, K-weights + x first.
"""

import math
import numpy as np
import ml_dtypes

import concourse.bass as bass
import concourse.bacc as bacc
import concourse.mybir as mybir
from concourse import tile
from concourse.bass_utils import run_bass_kernel_spmd

C = 768
H = 12
HD = 64
CB = C // 128           # feature blocks of 128
CBP = CB // 2           # feature pair-blocks of 256 (DoubleRow)
F16 = mybir.dt.float16
F32 = mybir.dt.float32
F8 = mybir.dt.float8e4
DR = mybir.MatmulPerfMode.DoubleRow
NPF8 = ml_dtypes.float8_e4m3

# fp8 range scales (see docstring)
AQ = 32.0               # Wq (with 1/sqrt(hd) folded) scale
AK = 32.0               # Wk scale
AV = 8.0                # Wv scale
AP_ = 32.0              # Wproj scale
ONES = 0.5              # denominator ones-column value
S_SCALE = 1.0 / (AQ * AK * 8.0)   # S_psum -> true S (1/sqrt(64) fold in AQ)
O_SCALE = AV / ONES               # OAT = O_SCALE * O_true
PROJ_SCALE = 1.0 / (O_SCALE * AP_)
KILL = -30000.0

_cache = {}


def _groups(n, limit=512):
    out = []
    off = 0
    while off < n:
        g = min(limit, n - off)
        out.append((off, g))
        off += g
    return out


def _build(NK, ND, NKM):
    """Build + bacc-compile the 8-core SPMD program for padded sizes."""
    KB = NK // 128
    KBP = KB // 2
    nc = bacc.Bacc("TRN2", target_bir_lowering=False, debug=False,
                   num_devices=8)

    wk8 = nc.dram_tensor("wk8", [CBP * 128, 2 * C], F8, kind="ExternalInput").ap()
    wq8 = nc.dram_tensor("wq8", [CBP * 128, 2 * C], F8, kind="ExternalInput").ap()
    wv8 = nc.dram_tensor("wv8", [CBP * 128, 2 * C], F8, kind="ExternalInput").ap()
    xc8 = nc.dram_tensor("xc8", [CBP * 128, 2 * NK], F8, kind="ExternalInput").ap()
    wp8 = nc.dram_tensor("wp8", [CBP * 128, 2 * C], F8, kind="ExternalInput").ap()
    xdT = nc.dram_tensor("xdT", [C, ND], F16, kind="ExternalInput").ap()
    w2T = nc.dram_tensor("w2T", [C, C], F16, kind="ExternalInput").ap()
    biasT = nc.dram_tensor("biasT", [128, CB], F32, kind="ExternalInput").ap()
    killT = nc.dram_tensor("killT", [128, KB], F32, kind="ExternalInput").ap()
    outkT = nc.dram_tensor("outkT", [C, NKM], F16, kind="ExternalOutput").ap()
    outdT = nc.dram_tensor("outdT", [C, ND], F16, kind="ExternalOutput").ap()
    import os
    DBG = bool(os.environ.get("KDBG"))
    if DBG:
        dq0 = nc.dram_tensor("dq0", [128, NKM], F16, kind="ExternalOutput").ap()
        dk0 = nc.dram_tensor("dk0", [128, NK], F16, kind="ExternalOutput").ap()
        de0 = nc.dram_tensor("de0", [128, 2 * NKM], F8, kind="ExternalOutput").ap()
        dv0 = nc.dram_tensor("dv0", [128, H * 256], F8, kind="ExternalOutput").ap()
        do0 = nc.dram_tensor("do0", [128, 2 * NKM], F8, kind="ExternalOutput").ap()

    GK = _groups(NK)
    GKM = _groups(NKM)
    GC = _groups(C)

    with tile.TileContext(nc) as tc:
        with (
            tc.tile_pool(name="const", bufs=1) as cpool,
            tc.tile_pool(name="ins", bufs=1) as ipool,
            tc.tile_pool(name="acts", bufs=1) as apool,
            tc.tile_pool(name="work", bufs=4) as wpool,
            tc.tile_pool(name="outs", bufs=3) as opool,
            tc.tile_pool(name="ps", bufs=2, space="PSUM") as pspool,
            tc.tile_pool(name="pt", bufs=2, space="PSUM") as ptpool,
        ):
            # ---- merged inputs (DMA emission order = priority) ----
            def dma_blocked(dst, src, nb):
                nc.sync.dma_start(
                    dst[:].rearrange("p (b c) -> p b c", b=nb),
                    src.rearrange("(b p) c -> p b c", p=128))

            wk_t = ipool.tile([128, CBP * 2 * C], F8, name="wk", tag="wk")
            xc_t = ipool.tile([128, CBP * 2 * NK], F8, name="xc", tag="xc")
            for bp in range(CBP):
                nc.sync.dma_start(wk_t[:, bp * 2 * C:(bp + 1) * 2 * C],
                                  wk8[bp * 128:(bp + 1) * 128, :])
                nc.sync.dma_start(xc_t[:, bp * 2 * NK:(bp + 1) * 2 * NK],
                                  xc8[bp * 128:(bp + 1) * 128, :])
            wqt_t = ipool.tile([128, CBP * 2 * C], F8, name="wq", tag="wq")
            dma_blocked(wqt_t, wq8, CBP)
            wv_t = ipool.tile([128, CBP * 2 * C], F8, name="wv", tag="wv")
            dma_blocked(wv_t, wv8, CBP)
            kill_t = cpool.tile([128, KB], F32, name="kill", tag="kill")
            nc.sync.dma_start(kill_t[:], killT[:])
            xd_t = ipool.tile([128, CB * ND], F16, name="xd", tag="xd")
            dma_blocked(xd_t, xdT, CB)
            w2_t = ipool.tile([128, CB * C], F16, name="w2", tag="w2")
            dma_blocked(w2_t, w2T, CB)
            wp_t = ipool.tile([128, CBP * 2 * C], F8, name="wp", tag="wp")
            dma_blocked(wp_t, wp8, CBP)
            bias_t = cpool.tile([128, CB], F32, name="bias", tag="bias")
            nc.sync.dma_start(bias_t[:], biasT[:])

            def w_slice(t, bp, j):
                """[128, 2, 128] DoubleRow lhsT slice: pair-block bp, col
                chunk j of a [128, CBP*2*C] fp8 weight tile."""
                return t[:, bp * 2 * C:(bp + 1) * 2 * C].rearrange(
                    "p (s c) -> p s c", s=2)[:, :, j * 128:(j + 1) * 128]

            def x_slice(bp, o, n):
                return xc_t[:, bp * 2 * NK:(bp + 1) * 2 * NK].rearrange(
                    "p (s c) -> p s c", s=2)[:, :, o:o + n]

            def xw_slice(bp, tb):
                """x as DoubleRow lhsT for the V (token-major) matmul."""
                return xc_t[:, bp * 2 * NK:(bp + 1) * 2 * NK].rearrange(
                    "p (s c) -> p s c", s=2)[:, :, tb * 128:(tb + 1) * 128]

            def wv_rhs(bp, o, n):
                return wv_t[:, bp * 2 * C:(bp + 1) * 2 * C].rearrange(
                    "p (s c) -> p s c", s=2)[:, :, o:o + n]

            # ---- persistent intermediates ----
            QcT = [apool.tile([128, NKM], F16, name=f"q{j}", tag=f"q{j}")
                   for j in range(CB)]
            KcT = [apool.tile([128, NK], F16, name=f"k{j}", tag=f"k{j}")
                   for j in range(CB)]
            Vag = [apool.tile([128, H * 256], F8, name=f"va{kp}",
                              tag=f"va{kp}") for kp in range(KBP)]
            for kp in range(KBP):
                va4 = Vag[kp][:].rearrange("p (h s m) -> p h s m", s=2, m=128)
                nc.gpsimd.memset(va4[:, :, :, 0:HD], ONES)
            OAT = [apool.tile([128, 2 * NKM], F8, name=f"oat{fp}",
                              tag=f"oat{fp}") for fp in range(CBP)]
            ET = {}
            for hm in range(6):
                for kp in range(KBP):
                    ET[(hm, kp)] = apool.tile(
                        [128, 2 * NKM], F8, name=f"et{hm}_{kp}",
                        tag=f"et{hm}_{kp}")

            def qkv_units(j):
                """f-major chunk j of Wq/Wk (whole chunk = one unit so the
                shared psum buf is alloc'd and released within the unit)."""
                grps = GKM if j < CB else GK
                w = NKM if j < CB else NK
                wt = wqt_t if j < CB else wk_t

                def chunk():
                    ps = pspool.tile([128, w], F32, name="qps", tag="s")
                    for bp in range(CBP):
                        for (o, n) in grps:
                            nc.tensor.matmul(
                                ps[:, o:o + n],
                                lhsT=w_slice(wt, bp, j % CB),
                                rhs=x_slice(bp, o, n),
                                start=(bp == 0), stop=(bp == CBP - 1),
                                perf_mode=DR)
                    dest = QcT[j] if j < CB else KcT[j - CB]
                    nc.vector.tensor_copy(dest[:, 0:w], ps[:, 0:w])

                return [chunk]

            def v_units(tb):
                """token-major V chunk for kept block tb (one unit)."""
                def chunk():
                    ps = pspool.tile([128, C], F32, name="vps", tag="s")
                    for bp in range(CBP):
                        for (o, n) in GC:
                            nc.tensor.matmul(
                                ps[:, o:o + n],
                                lhsT=xw_slice(bp, tb),
                                rhs=wv_rhs(bp, o, n),
                                start=(bp == 0), stop=(bp == CBP - 1),
                                perf_mode=DR)
                    va4 = Vag[tb // 2][:].rearrange(
                        "p (h s m) -> p h s m", s=2, m=128)
                    ps3 = ps[:].rearrange("p (h d) -> p h d", d=HD)
                    nc.vector.tensor_copy(va4[:, :, tb % 2, HD:128], ps3)

                return [chunk]

            def s_exp_kb_h(p, kb, hh):
                """S^T then exp->fp8 for head 2p+hh at key block kb."""
                fc = p
                h = 2 * p + hh
                rows = slice(hh * 64, hh * 64 + 64)
                et = ET[(h % 6, kb // 2)]
                ps = pspool.tile([128, NKM], F32, name="sps", tag="s")
                for (o, n) in GKM:
                    nc.tensor.matmul(
                        ps[:, o:o + n],
                        lhsT=KcT[fc][rows, kb * 128:(kb + 1) * 128],
                        rhs=QcT[fc][rows, o:o + n],
                        start=True, stop=True)
                nc.scalar.activation(
                    et[:, (kb % 2) * NKM:(kb % 2) * NKM + NKM], ps[:],
                    mybir.ActivationFunctionType.Exp,
                    bias=kill_t[:, kb:kb + 1], scale=S_SCALE)

            def proj_kept(cb):
                """out^T[c-block cb, :] = (OAT @ wp) * PROJ_SCALE + bias."""
                ps = pspool.tile([128, NKM], F32, name="pps", tag="s")
                for fp in range(CBP):
                    oat3 = OAT[fp][:].rearrange("p (s t) -> p s t", s=2)
                    for (o, n) in GKM:
                        nc.tensor.matmul(
                            ps[:, o:o + n],
                            lhsT=w_slice(wp_t, fp, cb),
                            rhs=oat3[:, :, o:o + n],
                            start=(fp == 0), stop=(fp == CBP - 1),
                            perf_mode=DR)
                ok = opool.tile([128, NKM], F16, name="ok", tag="ok")
                nc.vector.tensor_scalar(
                    ok[:], ps[:], PROJ_SCALE, bias_t[:, cb:cb + 1],
                    op0=mybir.AluOpType.mult, op1=mybir.AluOpType.add)
                nc.sync.dma_start(outkT[cb * 128:(cb + 1) * 128, :], ok[:])

            def proj_drop(cb):
                """out^T[c-block cb, :] = x_d @ W2^T + bias (fp16)."""
                ps = pspool.tile([128, ND], F32, name="dps", tag="s")
                for fb in range(CB):
                    nc.tensor.matmul(
                        ps[:],
                        lhsT=w2_t[:, fb * C + cb * 128:fb * C + (cb + 1) * 128],
                        rhs=xd_t[:, fb * ND:(fb + 1) * ND],
                        start=(fb == 0), stop=(fb == CB - 1))
                ok = opool.tile([128, ND], F16, name="od", tag="od")
                nc.vector.tensor_scalar(
                    ok[:], ps[:], bias_t[:, cb:cb + 1], None,
                    op0=mybir.AluOpType.add)
                nc.sync.dma_start(outdT[cb * 128:(cb + 1) * 128, :], ok[:])

            # ---- T^T accumulate + normalize, split into emission units ----
            # PSUM rows 0:64 hold the replicated denominator (ones block
            # first in Vag), rows 64:128 hold AV*T: reciprocal_approx_fast
            # reads PSUM at base partition 0 (base-64 custom reads are
            # broken), the multiply's in0 may be PSUM at base 64.
            def tpair_units(p):
                state = {}

                def accum(hh, kp):
                    h = 2 * p + hh
                    if kp == 0:
                        state[hh] = ptpool.tile([128, NKM], F32, name="ptT",
                                                tag="t2")
                    ptT = state[hh]
                    va = Vag[kp][:, h * 256:(h + 1) * 256].rearrange(
                        "p (s m) -> p s m", s=2)
                    et3 = ET[(h % 6, kp)][:].rearrange(
                        "p (s t) -> p s t", s=2)
                    for (o, n) in GKM:
                        nc.tensor.matmul(
                            ptT[:, o:o + n],
                            lhsT=va,
                            rhs=et3[:, :, o:o + n],
                            start=(kp == 0), stop=(kp == KBP - 1),
                            perf_mode=DR)

                def chain(hh):
                    h = 2 * p + hh
                    fp = h // 4
                    sl = (h // 2) % 2
                    orow = (h % 2) * 64
                    ptT = state[hh]
                    rb = wpool.tile([64, NKM], F32, name="rb", tag="rb")
                    nc.vector.reciprocal_approx_fast(rb[:], ptT[0:64, :])
                    nc.vector.tensor_tensor(
                        OAT[fp][orow:orow + 64, sl * NKM:sl * NKM + NKM],
                        ptT[64:128, :], rb[:], op=mybir.AluOpType.mult)

                units = []
                for hh in range(2):
                    units += [lambda hh=hh, kp=kp: accum(hh, kp)
                              for kp in range(KBP)]
                    units.append(lambda hh=hh: chain(hh))
                return units

            # ---- schedule ----
            # S/exp is the rate-limiting stream (ScalarE); other PE work is
            # woven between S slots in fine units so the PE never stalls
            # long (HAM stays at full clock). proj_drop runs in the tail so
            # its w2-DMA wait never blocks the PE FIFO behind early S work.
            NP = H // 2
            for f in qkv_units(CB + 0) + qkv_units(0):
                f()
            for p in range(NP):
                fillers = []
                if p + 1 < NP:
                    fillers += qkv_units(CB + p + 1) + qkv_units(p + 1)
                if p == 0:
                    for tb in range(KB):
                        fillers += v_units(tb)
                if p >= 1:
                    fillers += tpair_units(p - 1)
                nslots = 2 * KB
                done = 0
                slot = 0
                for kb in range(KB):
                    for hh in range(2):
                        s_exp_kb_h(p, kb, hh)
                        slot += 1
                        want = (len(fillers) * slot + nslots - 1) // nslots
                        while done < want:
                            fillers[done]()
                            done += 1
            tail = tpair_units(NP - 1)
            for cb in range(CB):
                tail.append(lambda cb=cb: proj_drop(cb))
            for f in tail:
                f()
            for cb in range(CB):
                proj_kept(cb)
            if DBG:
                nc.sync.dma_start(dq0[:], QcT[0][:])
                nc.sync.dma_start(dk0[:], KcT[0][:])
                nc.sync.dma_start(de0[:], ET[(0, 0)][:])
                nc.sync.dma_start(dv0[:], Vag[0][:])
                nc.sync.dma_start(do0[:], OAT[0][:])

    nc.compile()
    return nc


def kernel(x, policy, Wqkv, Wproj, bproj, _trace=False, _tmpdir=None):
    x = np.asarray(x)
    policy = np.asarray(policy)
    Wqkv = np.asarray(Wqkv, dtype=np.float32)
    Wproj = np.asarray(Wproj, dtype=np.float32)
    bproj = np.asarray(bproj, dtype=np.float32)
    B, N, _ = x.shape
    assert B == 8 and x.shape[2] == C

    pol = policy[:, :, 0] > 0.5
    kept = [np.nonzero(pol[b])[0] for b in range(B)]
    drop = [np.nonzero(~pol[b])[0] for b in range(B)]
    nk = [len(i) for i in kept]
    nd = [len(i) for i in drop]
    NK = max(256, int(math.ceil(max(nk) / 256.0)) * 256)
    ND = max(128, int(math.ceil(max(nd) / 128.0)) * 128)
    NKM = min(NK, max(128, int(math.ceil(max(nk) / 32.0)) * 32))
    KB = NK // 128
    assert NK - min(nk) <= 0x7FFF

    key = (NK, ND, NKM)
    if key not in _cache:
        _cache[key] = _build(NK, ND, NKM)
    nc = _cache[key]

    # ---- shared weight prep ----
    def dr_pack(wT, scale):
        # [C, cols] f-major -> DoubleRow pair layout [CBP*128, 2*cols]
        a = (wT * scale).astype(np.float32)
        cols = a.shape[1]
        a = a.reshape(CBP, 2, 128, cols).transpose(0, 2, 1, 3)
        return np.ascontiguousarray(a.reshape(CBP * 128, 2 * cols)).astype(NPF8)

    wqkvT = np.ascontiguousarray(Wqkv.T)           # [C, 3C]
    wq8a = dr_pack(wqkvT[:, 0:C], AQ)              # 1/sqrt(hd) lives in S_SCALE
    wk8a = dr_pack(wqkvT[:, C:2 * C], AK)
    wv8a = dr_pack(wqkvT[:, 2 * C:3 * C], AV)
    wp8a = dr_pack(np.ascontiguousarray(Wproj.T), AP_)
    W2 = Wproj @ Wqkv[2 * C:3 * C]
    w2Ta = np.ascontiguousarray(W2.T).astype(np.float16)
    biasa = np.ascontiguousarray(
        bproj.reshape(CB, 128).T).astype(np.float32)   # [128, CB]

    in_maps = []
    for b in range(B):
        xcT = np.zeros((C, NK), np.float32)
        xcT[:, :nk[b]] = x[b][kept[b]].T
        xc8a = np.ascontiguousarray(
            xcT.reshape(CBP, 2, 128, NK).transpose(0, 2, 1, 3)
            .reshape(CBP * 128, 2 * NK)).astype(NPF8)
        xdTa = np.zeros((C, ND), np.float16)
        xdTa[:, :nd[b]] = x[b][drop[b]].T
        killa = np.zeros((128, KB), np.float32)
        for kb in range(KB):
            lo = kb * 128
            for p_ in range(128):
                if lo + p_ >= nk[b]:
                    killa[p_, kb] = KILL
        in_maps.append({
            "wk8": wk8a, "wq8": wq8a, "wv8": wv8a, "xc8": xc8a,
            "wp8": wp8a, "xdT": xdTa, "w2T": w2Ta, "biasT": biasa,
            "killT": killa,
        })

    res = run_bass_kernel_spmd(nc, in_maps, core_ids=list(range(B)),
                               trace=_trace, tmpdir=_tmpdir)

    out = np.empty((B, N, C), np.float32)
    for b in range(B):
        out[b, kept[b]] = res.results[b]["outkT"][:, :nk[b]].T.astype(np.float32)
        out[b, drop[b]] = res.results[b]["outdT"][:, :nd[b]].T.astype(np.float32)
    kernel._last = res
    return out


# revision 12
# speedup vs baseline: 1.0484x; 1.0342x over previous
"""Policy-masked sparse attention on 8 trn2 NeuronCores.

Strategy (data-parallel over B: one batch element per core):
  Reference softmax-with-policy (eps=1e-6) reduces, for this regime, to:
    - dropped queries (policy=0): out row = v_row @ Wproj + b  (x @ W2)
    - kept queries: out = (E @ V) / (E @ 1) over kept keys, E = exp(S)
  Host compacts kept/dropped tokens, pads kept to NK (mult of 256) and
  queries to NKM (mult of 32), pre-scales + fp8-casts weights.

  Device (per core):
    - QKV + V and the two projections run as fp8(e4m3) DoubleRow matmuls
      (contraction 256 per pass) — ~1.7x PE throughput vs fp16.
    - S^T = K^T.T @ Q^T per head (fp16, contraction 64), exp on ScalarE
      with scale=1/8192 folded in and a per-partition bias that kills
      padded keys (exp -> 0), output directly as fp8 into the DoubleRow
      slot layout.
    - T^T accumulation uses an augmented V: per head 64 v-columns + 64
      constant 0.5-columns, so PSUM rows 64:128 hold the softmax
      denominator replicated 64x. reciprocal_approx_fast on [64, NKM]
      then one tensor_tensor multiply writes the normalized attention
      output as fp8 (x16 scale) in proj DoubleRow layout. No gpsimd
      broadcast, no ScalarE copies.
    - proj is emitted output-transposed (out^T[c, t]) so the bias is a
      per-partition scalar folded into the DVE epilogue; host transposes.
  Input DMAs are merged into 9 large descriptors, K-weights + x first.
"""

import math
import numpy as np
import ml_dtypes

import concourse.bass as bass
import concourse.bacc as bacc
import concourse.mybir as mybir
from concourse import tile
from concourse.bass_utils import run_bass_kernel_spmd

C = 768
H = 12
HD = 64
CB = C // 128           # feature blocks of 128
CBP = CB // 2           # feature pair-blocks of 256 (DoubleRow)
F16 = mybir.dt.float16
F32 = mybir.dt.float32
F8 = mybir.dt.float8e4
DR = mybir.MatmulPerfMode.DoubleRow
NPF8 = ml_dtypes.float8_e4m3

# fp8 range scales (see docstring)
AQ = 32.0               # Wq (with 1/sqrt(hd) folded) scale
AK = 32.0               # Wk scale
AV = 8.0                # Wv scale
AP_ = 32.0              # Wproj scale
ONES = 0.5              # denominator ones-column value
S_SCALE = 1.0 / (AQ * AK * 8.0)   # S_psum -> true S (1/sqrt(64) fold in AQ)
O_SCALE = AV / ONES               # OAT = O_SCALE * O_true
PROJ_SCALE = 1.0 / (O_SCALE * AP_)
KILL = -30000.0

_cache = {}


def _groups(n, limit=512):
    out = []
    off = 0
    while off < n:
        g = min(limit, n - off)
        out.append((off, g))
        off += g
    return out


def _build(NK, ND, NKM):
    """Build + bacc-compile the 8-core SPMD program for padded sizes."""
    KB = NK // 128
    KBP = KB // 2
    nc = bacc.Bacc("TRN2", target_bir_lowering=False, debug=False,
                   num_devices=8)

    wk8 = nc.dram_tensor("wk8", [CBP * 128, 2 * C], F8, kind="ExternalInput").ap()
    wq8 = nc.dram_tensor("wq8", [CBP * 128, 2 * C], F8, kind="ExternalInput").ap()
    wv8 = nc.dram_tensor("wv8", [CBP * 128, 2 * C], F8, kind="ExternalInput").ap()
    xc8 = nc.dram_tensor("xc8", [CBP * 128, 2 * NK], F8, kind="ExternalInput").ap()
    wp8 = nc.dram_tensor("wp8", [CBP * 128, 2 * C], F8, kind="ExternalInput").ap()
    xdT = nc.dram_tensor("xdT", [C, ND], F16, kind="ExternalInput").ap()
    w2T = nc.dram_tensor("w2T", [C, C], F16, kind="ExternalInput").ap()
    biasT = nc.dram_tensor("biasT", [128, CB], F32, kind="ExternalInput").ap()
    killT = nc.dram_tensor("killT", [128, KB], F32, kind="ExternalInput").ap()
    outkT = nc.dram_tensor("outkT", [C, NKM], F16, kind="ExternalOutput").ap()
    outdT = nc.dram_tensor("outdT", [C, ND], F16, kind="ExternalOutput").ap()
    import os
    DBG = bool(os.environ.get("KDBG"))
    if DBG:
        dq0 = nc.dram_tensor("dq0", [128, NKM], F16, kind="ExternalOutput").ap()
        dk0 = nc.dram_tensor("dk0", [128, NK], F16, kind="ExternalOutput").ap()
        de0 = nc.dram_tensor("de0", [128, 2 * NKM], F8, kind="ExternalOutput").ap()
        dv0 = nc.dram_tensor("dv0", [128, H * 256], F8, kind="ExternalOutput").ap()
        do0 = nc.dram_tensor("do0", [128, 2 * NKM], F8, kind="ExternalOutput").ap()

    GK = _groups(NK)
    GKM = _groups(NKM)
    GC = _groups(C)

    with tile.TileContext(nc) as tc:
        with (
            tc.tile_pool(name="const", bufs=1) as cpool,
            tc.tile_pool(name="ins", bufs=1) as ipool,
            tc.tile_pool(name="acts", bufs=1) as apool,
            tc.tile_pool(name="work", bufs=4) as wpool,
            tc.tile_pool(name="outs", bufs=3) as opool,
            tc.tile_pool(name="ps", bufs=2, space="PSUM") as pspool,
            tc.tile_pool(name="pt", bufs=2, space="PSUM") as ptpool,
        ):
            # ---- merged inputs (DMA emission order = priority) ----
            def dma_blocked(dst, src, nb):
                nc.sync.dma_start(
                    dst[:].rearrange("p (b c) -> p b c", b=nb),
                    src.rearrange("(b p) c -> p b c", p=128))

            wk_t = ipool.tile([128, CBP * 2 * C], F8, name="wk", tag="wk")
            xc_t = ipool.tile([128, CBP * 2 * NK], F8, name="xc", tag="xc")
            nc.sync.dma_start(wk_t[:, 0:2 * C], wk8[0:128, :])
            nc.sync.dma_start(xc_t[:, 0:2 * NK], xc8[0:128, :])
            kill_t = cpool.tile([128, KB], F32, name="kill", tag="kill")
            nc.sync.dma_start(kill_t[:], killT[:])
            bias_t = cpool.tile([128, CB], F32, name="bias", tag="bias")
            nc.sync.dma_start(bias_t[:], biasT[:])
            for bp in range(1, CBP):
                nc.sync.dma_start(wk_t[:, bp * 2 * C:(bp + 1) * 2 * C],
                                  wk8[bp * 128:(bp + 1) * 128, :])
                nc.sync.dma_start(xc_t[:, bp * 2 * NK:(bp + 1) * 2 * NK],
                                  xc8[bp * 128:(bp + 1) * 128, :])
            wqt_t = ipool.tile([128, CBP * 2 * C], F8, name="wq", tag="wq")
            dma_blocked(wqt_t, wq8, CBP)
            wv_t = ipool.tile([128, CBP * 2 * C], F8, name="wv", tag="wv")
            dma_blocked(wv_t, wv8, CBP)
            xd_t = ipool.tile([128, CB * ND], F16, name="xd", tag="xd")
            dma_blocked(xd_t, xdT, CB)
            w2_t = ipool.tile([128, CB * C], F16, name="w2", tag="w2")
            dma_blocked(w2_t, w2T, CB)
            wp_t = ipool.tile([128, CBP * 2 * C], F8, name="wp", tag="wp")
            dma_blocked(wp_t, wp8, CBP)

            def w_slice(t, bp, j):
                """[128, 2, 128] DoubleRow lhsT slice: pair-block bp, col
                chunk j of a [128, CBP*2*C] fp8 weight tile."""
                return t[:, bp * 2 * C:(bp + 1) * 2 * C].rearrange(
                    "p (s c) -> p s c", s=2)[:, :, j * 128:(j + 1) * 128]

            def x_slice(bp, o, n):
                return xc_t[:, bp * 2 * NK:(bp + 1) * 2 * NK].rearrange(
                    "p (s c) -> p s c", s=2)[:, :, o:o + n]

            def xw_slice(bp, tb):
                """x as DoubleRow lhsT for the V (token-major) matmul."""
                return xc_t[:, bp * 2 * NK:(bp + 1) * 2 * NK].rearrange(
                    "p (s c) -> p s c", s=2)[:, :, tb * 128:(tb + 1) * 128]

            def wv_rhs(bp, o, n):
                return wv_t[:, bp * 2 * C:(bp + 1) * 2 * C].rearrange(
                    "p (s c) -> p s c", s=2)[:, :, o:o + n]

            # ---- persistent intermediates ----
            QcT = [apool.tile([128, NKM], F16, name=f"q{j}", tag=f"q{j}")
                   for j in range(CB)]
            KcT = [apool.tile([128, NK], F16, name=f"k{j}", tag=f"k{j}")
                   for j in range(CB)]
            Vag = [apool.tile([128, H * 256], F8, name=f"va{kp}",
                              tag=f"va{kp}") for kp in range(KBP)]
            for kp in range(KBP):
                va4 = Vag[kp][:].rearrange("p (h s m) -> p h s m", s=2, m=128)
                nc.gpsimd.memset(va4[:, :, :, 0:HD], ONES)
            OAT = [apool.tile([128, 2 * NKM], F8, name=f"oat{fp}",
                              tag=f"oat{fp}") for fp in range(CBP)]
            ET = {}
            for hm in range(6):
                for kp in range(KBP):
                    ET[(hm, kp)] = apool.tile(
                        [128, 2 * NKM], F8, name=f"et{hm}_{kp}",
                        tag=f"et{hm}_{kp}")

            def qkv_units(j):
                """f-major chunk j of Wq/Wk (whole chunk = one unit so the
                shared psum buf is alloc'd and released within the unit)."""
                grps = GKM if j < CB else GK
                w = NKM if j < CB else NK
                wt = wqt_t if j < CB else wk_t

                def chunk():
                    ps = pspool.tile([128, w], F32, name="qps", tag="s")
                    for bp in range(CBP):
                        for (o, n) in grps:
                            nc.tensor.matmul(
                                ps[:, o:o + n],
                                lhsT=w_slice(wt, bp, j % CB),
                                rhs=x_slice(bp, o, n),
                                start=(bp == 0), stop=(bp == CBP - 1),
                                perf_mode=DR)
                    dest = QcT[j] if j < CB else KcT[j - CB]
                    nc.vector.tensor_copy(dest[:, 0:w], ps[:, 0:w])

                return [chunk]

            def v_units(tb):
                """token-major V chunk for kept block tb (one unit)."""
                def chunk():
                    ps = pspool.tile([128, C], F32, name="vps", tag="s")
                    for bp in range(CBP):
                        for (o, n) in GC:
                            nc.tensor.matmul(
                                ps[:, o:o + n],
                                lhsT=xw_slice(bp, tb),
                                rhs=wv_rhs(bp, o, n),
                                start=(bp == 0), stop=(bp == CBP - 1),
                                perf_mode=DR)
                    va4 = Vag[tb // 2][:].rearrange(
                        "p (h s m) -> p h s m", s=2, m=128)
                    ps3 = ps[:].rearrange("p (h d) -> p h d", d=HD)
                    nc.vector.tensor_copy(va4[:, :, tb % 2, HD:128], ps3)

                return [chunk]

            def s_exp_kb_h(p, kb, hh):
                """S^T then exp->fp8 for head 2p+hh at key block kb."""
                fc = p
                h = 2 * p + hh
                rows = slice(hh * 64, hh * 64 + 64)
                et = ET[(h % 6, kb // 2)]
                ps = pspool.tile([128, NKM], F32, name="sps", tag="s")
                for (o, n) in GKM:
                    nc.tensor.matmul(
                        ps[:, o:o + n],
                        lhsT=KcT[fc][rows, kb * 128:(kb + 1) * 128],
                        rhs=QcT[fc][rows, o:o + n],
                        start=True, stop=True)
                nc.scalar.activation(
                    et[:, (kb % 2) * NKM:(kb % 2) * NKM + NKM], ps[:],
                    mybir.ActivationFunctionType.Exp,
                    bias=kill_t[:, kb:kb + 1], scale=S_SCALE)

            def proj_kept(cb):
                """out^T[c-block cb, :] = (OAT @ wp) * PROJ_SCALE + bias."""
                ps = pspool.tile([128, NKM], F32, name="pps", tag="s")
                for fp in range(CBP):
                    oat3 = OAT[fp][:].rearrange("p (s t) -> p s t", s=2)
                    for (o, n) in GKM:
                        nc.tensor.matmul(
                            ps[:, o:o + n],
                            lhsT=w_slice(wp_t, fp, cb),
                            rhs=oat3[:, :, o:o + n],
                            start=(fp == 0), stop=(fp == CBP - 1),
                            perf_mode=DR)
                ok = opool.tile([128, NKM], F16, name="ok", tag="ok")
                nc.vector.tensor_scalar(
                    ok[:], ps[:], PROJ_SCALE, bias_t[:, cb:cb + 1],
                    op0=mybir.AluOpType.mult, op1=mybir.AluOpType.add)
                nc.sync.dma_start(outkT[cb * 128:(cb + 1) * 128, :], ok[:])

            def proj_drop(cb):
                """out^T[c-block cb, :] = x_d @ W2^T + bias (fp16)."""
                ps = pspool.tile([128, ND], F32, name="dps", tag="s")
                for fb in range(CB):
                    nc.tensor.matmul(
                        ps[:],
                        lhsT=w2_t[:, fb * C + cb * 128:fb * C + (cb + 1) * 128],
                        rhs=xd_t[:, fb * ND:(fb + 1) * ND],
                        start=(fb == 0), stop=(fb == CB - 1))
                ok = opool.tile([128, ND], F16, name="od", tag="od")
                nc.vector.tensor_scalar(
                    ok[:], ps[:], bias_t[:, cb:cb + 1], None,
                    op0=mybir.AluOpType.add)
                nc.sync.dma_start(outdT[cb * 128:(cb + 1) * 128, :], ok[:])

            # ---- T^T accumulate + normalize, split into emission units ----
            # PSUM rows 0:64 hold the replicated denominator (ones block
            # first in Vag), rows 64:128 hold AV*T: reciprocal_approx_fast
            # reads PSUM at base partition 0 (base-64 custom reads are
            # broken), the multiply's in0 may be PSUM at base 64.
            def tpair_units(p):
                state = {}

                def accum(hh, kp):
                    h = 2 * p + hh
                    if kp == 0:
                        state[hh] = ptpool.tile([128, NKM], F32, name="ptT",
                                                tag="t2")
                    ptT = state[hh]
                    va = Vag[kp][:, h * 256:(h + 1) * 256].rearrange(
                        "p (s m) -> p s m", s=2)
                    et3 = ET[(h % 6, kp)][:].rearrange(
                        "p (s t) -> p s t", s=2)
                    for (o, n) in GKM:
                        nc.tensor.matmul(
                            ptT[:, o:o + n],
                            lhsT=va,
                            rhs=et3[:, :, o:o + n],
                            start=(kp == 0), stop=(kp == KBP - 1),
                            perf_mode=DR)

                def chain(hh):
                    h = 2 * p + hh
                    fp = h // 4
                    sl = (h // 2) % 2
                    orow = (h % 2) * 64
                    ptT = state[hh]
                    rb = wpool.tile([64, NKM], F32, name="rb", tag="rb")
                    nc.vector.reciprocal_approx_fast(rb[:], ptT[0:64, :])
                    nc.vector.tensor_tensor(
                        OAT[fp][orow:orow + 64, sl * NKM:sl * NKM + NKM],
                        ptT[64:128, :], rb[:], op=mybir.AluOpType.mult)

                units = []
                for hh in range(2):
                    units += [lambda hh=hh, kp=kp: accum(hh, kp)
                              for kp in range(KBP)]
                    units.append(lambda hh=hh: chain(hh))
                return units

            # ---- schedule ----
            # S/exp is the rate-limiting stream (ScalarE); other PE work is
            # woven between S slots in fine units so the PE never stalls
            # long (HAM stays at full clock). proj_drop fills pairs 4-5 and
            # the tail (its w2 DMA has long landed by then).
            NP = H // 2
            # Warm-up: PE runs dummy matmuls during the DMA head so the HAM
            # clock-gate reaches 2.4 GHz before real work; a dummy exp
            # preloads the ACT table set (~2.7us) off the critical path.
            warm_sb = cpool.tile([128, 640], F16, name="warm", tag="warm")
            nc.gpsimd.memset(warm_sb[:], 0.0)
            warm_ps = ptpool.tile([128, 512], F32, name="wps", tag="t2")
            for _ in range(18):
                nc.tensor.matmul(warm_ps[:], lhsT=warm_sb[:, 512:640],
                                 rhs=warm_sb[:, 0:512], start=True, stop=True)
            warm_act = cpool.tile([1, 32], F32, name="wact", tag="wact")
            nc.gpsimd.memset(warm_act[:], 0.0)
            nc.scalar.activation(warm_act[:], warm_act[:],
                                 mybir.ActivationFunctionType.Exp)
            for f in qkv_units(CB + 0) + qkv_units(0):
                f()
            for p in range(NP):
                fillers = []
                if p + 1 < NP:
                    fillers += qkv_units(CB + p + 1) + qkv_units(p + 1)
                if p == 0:
                    for tb in range(KB):
                        fillers += v_units(tb)
                if p >= 1:
                    fillers += tpair_units(p - 1)
                if p == NP - 2:
                    fillers += [lambda cb=cb: proj_drop(cb) for cb in (0, 1)]
                if p == NP - 1:
                    fillers += [lambda cb=cb: proj_drop(cb)
                                for cb in (2, 3, 4)]
                nslots = 2 * KB
                done = 0
                slot = 0
                for kb in range(KB):
                    for hh in range(2):
                        s_exp_kb_h(p, kb, hh)
                        slot += 1
                        want = (len(fillers) * slot + nslots - 1) // nslots
                        while done < want:
                            fillers[done]()
                            done += 1
            tail = tpair_units(NP - 1)
            tail.append(lambda: proj_drop(5))
            for f in tail:
                f()
            for cb in range(CB):
                proj_kept(cb)
            if DBG:
                nc.sync.dma_start(dq0[:], QcT[0][:])
                nc.sync.dma_start(dk0[:], KcT[0][:])
                nc.sync.dma_start(de0[:], ET[(0, 0)][:])
                nc.sync.dma_start(dv0[:], Vag[0][:])
                nc.sync.dma_start(do0[:], OAT[0][:])

    nc.compile()
    return nc


def kernel(x, policy, Wqkv, Wproj, bproj, _trace=False, _tmpdir=None):
    x = np.asarray(x)
    policy = np.asarray(policy)
    Wqkv = np.asarray(Wqkv, dtype=np.float32)
    Wproj = np.asarray(Wproj, dtype=np.float32)
    bproj = np.asarray(bproj, dtype=np.float32)
    B, N, _ = x.shape
    assert B == 8 and x.shape[2] == C

    pol = policy[:, :, 0] > 0.5
    kept = [np.nonzero(pol[b])[0] for b in range(B)]
    drop = [np.nonzero(~pol[b])[0] for b in range(B)]
    nk = [len(i) for i in kept]
    nd = [len(i) for i in drop]
    NK = max(256, int(math.ceil(max(nk) / 256.0)) * 256)
    ND = max(128, int(math.ceil(max(nd) / 128.0)) * 128)
    NKM = min(NK, max(128, int(math.ceil(max(nk) / 32.0)) * 32))
    KB = NK // 128
    assert NK - min(nk) <= 0x7FFF

    key = (NK, ND, NKM)
    if key not in _cache:
        _cache[key] = _build(NK, ND, NKM)
    nc = _cache[key]

    # ---- shared weight prep ----
    def dr_pack(wT, scale):
        # [C, cols] f-major -> DoubleRow pair layout [CBP*128, 2*cols]
        a = (wT * scale).astype(np.float32)
        cols = a.shape[1]
        a = a.reshape(CBP, 2, 128, cols).transpose(0, 2, 1, 3)
        return np.ascontiguousarray(a.reshape(CBP * 128, 2 * cols)).astype(NPF8)

    wqkvT = np.ascontiguousarray(Wqkv.T)           # [C, 3C]
    wq8a = dr_pack(wqkvT[:, 0:C], AQ)              # 1/sqrt(hd) lives in S_SCALE
    wk8a = dr_pack(wqkvT[:, C:2 * C], AK)
    wv8a = dr_pack(wqkvT[:, 2 * C:3 * C], AV)
    wp8a = dr_pack(np.ascontiguousarray(Wproj.T), AP_)
    W2 = Wproj @ Wqkv[2 * C:3 * C]
    w2Ta = np.ascontiguousarray(W2.T).astype(np.float16)
    biasa = np.ascontiguousarray(
        bproj.reshape(CB, 128).T).astype(np.float32)   # [128, CB]

    in_maps = []
    for b in range(B):
        xcT = np.zeros((C, NK), np.float32)
        xcT[:, :nk[b]] = x[b][kept[b]].T
        xc8a = np.ascontiguousarray(
            xcT.reshape(CBP, 2, 128, NK).transpose(0, 2, 1, 3)
            .reshape(CBP * 128, 2 * NK)).astype(NPF8)
        xdTa = np.zeros((C, ND), np.float16)
        xdTa[:, :nd[b]] = x[b][drop[b]].T
        killa = np.zeros((128, KB), np.float32)
        for kb in range(KB):
            lo = kb * 128
            for p_ in range(128):
                if lo + p_ >= nk[b]:
                    killa[p_, kb] = KILL
        in_maps.append({
            "wk8": wk8a, "wq8": wq8a, "wv8": wv8a, "xc8": xc8a,
            "wp8": wp8a, "xdT": xdTa, "w2T": w2Ta, "biasT": biasa,
            "killT": killa,
        })

    res = run_bass_kernel_spmd(nc, in_maps, core_ids=list(range(B)),
                               trace=_trace, tmpdir=_tmpdir)

    out = np.empty((B, N, C), np.float32)
    for b in range(B):
        out[b, kept[b]] = res.results[b]["outkT"][:, :nk[b]].T.astype(np.float32)
        out[b, drop[b]] = res.results[b]["outdT"][:, :nd[b]].T.astype(np.float32)
    kernel._last = res
    return out
